# revision 50
# baseline (speedup 1.0000x reference)
"""Multi-head causal attention (B=2, S=2048, D=1024, H=16) on 8 NeuronCores.

Sharding: core c handles batch c//4 and head group c%4 (4 heads = 256 dims).
Wq/Wk/Wv are column-split by head (each core projects only its heads);
Wo is row-split; the Wo all-reduce is done on the host by summing the 8
partial outputs (4 cores per batch), then adding bo.

Per-core device program (all matmuls in bf16, accumulation in f32 PSUM):
  A) Q/K projections -> Q^T, K^T laid out (head_dim, seq) in SBUF;
     V projection -> natural (seq, head_dim) with a ones column appended
     (so attn@V also produces the softmax row-sums as a 65th output row).
  B) Per (head, q-half): scores computed transposed (k on partitions,
     q on free dim) -> exp on ScalarE (no max subtraction; scores ~N(0,1))
     -> attn@V accumulating O^T (65, q) in PSUM. Causal masking = skip
     fully-masked column ranges in the matmuls + one triangular 128x128
     mask multiply per diagonal k-tile. Normalization: DVE fast reciprocal
     of the PSUM row-sum row -> rank-1 matmul (ones[1,64] x rec[1,1024])
     broadcasts it across partitions in PSUM -> DVE multiply.
  C) Output projection from the accumulated A^T (256, seq) against
     Wo^T slice, streamed out per 128-row tile as bf16.
"""

import ml_dtypes
import numpy as np

import concourse.bass as bass
import concourse.tile as tile
from concourse import bacc, mybir
from concourse.bass_utils import run_bass_kernel_spmd

B, S, D, H = 2, 2048, 1024, 16
DH = D // H          # 64
HPC = 4              # heads per core
HD = HPC * DH        # 256 head dims per core
N_CORES = 8
DT = D // 128        # 8 contraction tiles for projections
NT = S // 128        # 16 seq tiles
F32 = mybir.dt.float32
F32R = mybir.dt.float32r
BF16 = mybir.dt.bfloat16
F8 = mybir.dt.float8e4
DR = mybir.MatmulPerfMode.DoubleRow


def build_program(dbg=False, num_devices=N_CORES):
    nc = bacc.Bacc("TRN2", target_bir_lowering=False, debug=False,
                   num_devices=num_devices)

    qT_d = nc.dram_tensor("qT", [D, S], BF16, kind="ExternalInput").ap()
    kT_d = nc.dram_tensor("kT", [D, S], BF16, kind="ExternalInput").ap()
    vT_d = nc.dram_tensor("vT", [D, S], BF16, kind="ExternalInput").ap()
    wq_d = nc.dram_tensor("wq", [D, HD], BF16, kind="ExternalInput").ap()
    wk_d = nc.dram_tensor("wk", [D, HD], BF16, kind="ExternalInput").ap()
    wv_d = nc.dram_tensor("wv", [D, HD], BF16, kind="ExternalInput").ap()
    wo_d = nc.dram_tensor("wo", [HD, D], BF16, kind="ExternalInput").ap()
    tri_d = nc.dram_tensor("tri", [128, 128], BF16, kind="ExternalInput").ap()
    ones_d = nc.dram_tensor("ones", [128, 64], BF16,
                            kind="ExternalInput").ap()
    # DRAM staging for the reciprocal rowsum rows: a stride-0 partition
    # broadcast is only legal when reading from DRAM.
    rrec_d = nc.dram_tensor("rrec", [8, 1024], F32,
                            kind="ExternalOutput" if dbg else "Internal").ap()
    y_d = nc.dram_tensor("y", [S, D], BF16, kind="ExternalOutput").ap()
    if dbg:
        dqT_d = nc.dram_tensor("dqT", [128, 2, S], BF16, kind="ExternalOutput").ap()
        dkT_d = nc.dram_tensor("dkT", [128, 2, S], BF16, kind="ExternalOutput").ap()
        dvo_d = nc.dram_tensor("dvo", [128, NT, HPC, DH + 1], BF16,
                               kind="ExternalOutput").ap()
        daT_d = nc.dram_tensor("daT", [128, 2, S], BF16, kind="ExternalOutput").ap()
        dst_d = nc.dram_tensor("dst", [128, 1024], F32, kind="ExternalOutput").ap()
        dpT_d = nc.dram_tensor("dpT", [128, 1024], BF16, kind="ExternalOutput").ap()
        doT_d = nc.dram_tensor("doT", [2, 65, 1024], F32, kind="ExternalOutput").ap()

    Exp = mybir.ActivationFunctionType.Exp

    with tile.TileContext(nc) as tc:
        with (
            tc.tile_pool(name="persist", bufs=1) as persist,
            tc.tile_pool(name="stream", bufs=4) as qstream,
            tc.tile_pool(name="kstream", bufs=4) as kstream,
            tc.tile_pool(name="vstream", bufs=8) as vstream,
            tc.tile_pool(name="pT", bufs=6) as pT_pool,
            tc.tile_pool(name="norm", bufs=2) as norm_pool,
            tc.tile_pool(name="ysb", bufs=4) as ysb_pool,
        ):
            # ---- constants / weights (one DMA each, spread over queues) ----
            # All matmuls bf16: fp8 anywhere in these dot-product chains
            # costs ~4% relative error (random-sign sums don't average the
            # quantization noise away) and fails the accuracy gate.
            wq_sb = persist.tile([128, DT, HD], BF16, tag="wq")
            wk_sb = persist.tile([128, DT, HD], BF16, tag="wk")
            wv_sb = persist.tile([128, DT, HD], BF16, tag="wv")
            wo_sb = persist.tile([128, 2, D], BF16, tag="wo")
            tri_sb = persist.tile([128, 128], BF16, tag="tri")
            ones_sb = persist.tile([128, 64], BF16, tag="ones")

            # Weights on the scalar queue; x streams go on the sync queue
            # staggered q -> k -> v so each tensor gets full HBM bandwidth
            # just before its projection needs it (concurrent streams would
            # starve Q proj).
            # wq dt=0 slice first: the very first matmul needs only it, so
            # don't make it wait for the full 0.5MB load.
            wq_r = wq_d.rearrange("(t p) h -> p t h", p=128)
            nc.scalar.dma_start(wq_sb[:, 0:1, :], wq_r[:, 0:1, :])
            nc.scalar.dma_start(wq_sb[:, 1:, :], wq_r[:, 1:, :])
            nc.scalar.dma_start(wk_sb, wk_d.rearrange("(t p) h -> p t h", p=128))
            nc.scalar.dma_start(wv_sb, wv_d.rearrange("(t p) h -> p t h", p=128))
            nc.scalar.dma_start(tri_sb, tri_d)
            nc.scalar.dma_start(ones_sb, ones_d)
            nc.scalar.dma_start(wo_sb, wo_d.rearrange("(t p) e -> p t e", p=128))

            qT_sb = persist.tile([128, 2, S], BF16, tag="qTsb")
            kT_sb = persist.tile([128, 2, S], BF16, tag="kTsb")
            # V with ones column: [128, seq_tile, head, 65]
            vo_sb = persist.tile([128, NT, HPC, DH + 1], BF16, tag="vones")
            nc.vector.tensor_copy(
                vo_sb[:, :, :, DH:DH + 1],
                ones_sb.rearrange("p (t h one) -> p t h one", h=HPC, one=1))
            aT_sb = persist.tile([128, 2, S], BF16, tag="aTsb")

            # ---- Phase A: projections (fp8 DoubleRow: 2 contraction
            # ---- subtiles per pass, 0.5 cycles per moving column) ----
            with tc.tile_pool(name="psA", bufs=8, space="PSUM") as psA:
                def proj_qk(x_d, w_sb, dst_sb, pool):
                    ps = [psA.tile([128, 512], F32, tag="psA", name=f"psA{i}")
                          for i in range(8)]
                    for dt in range(DT):
                        xt = pool.tile([128, S], BF16)
                        (nc.sync if dt % 2 == 0 else nc.gpsimd).dma_start(
                            xt, x_d[dt * 128:(dt + 1) * 128, :])
                        for half in range(2):
                            for t in range(2):
                                for c in range(2):
                                    cc = half * 2 + c
                                    nc.tensor.matmul(
                                        ps[t * 4 + cc],
                                        w_sb[:, dt, t * 128:(t + 1) * 128],
                                        xt[:, half * 1024 + c * 512:
                                           half * 1024 + (c + 1) * 512],
                                        start=(dt == 0), stop=(dt == DT - 1),
                                    )
                    for t in range(2):
                        for c in range(4):
                            nc.vector.tensor_copy(
                                dst_sb[:, t, c * 512:(c + 1) * 512],
                                ps[t * 4 + c])

                proj_qk(qT_d, wq_sb, qT_sb, qstream)
                proj_qk(kT_d, wk_sb, kT_sb, kstream)

                # V: natural layout out[m = seq_tile(128), n = head dims(256)].
                # ntile-outer so each PSUM tile's copy issues right after its
                # accumulation matmuls (no bunched drain at the A->B edge).
                vts = []
                for dt in range(DT):
                    vt = vstream.tile([128, S], BF16)
                    (nc.sync if dt % 2 == 0 else nc.gpsimd).dma_start(
                        vt, vT_d[dt * 128:(dt + 1) * 128, :])
                    vts.append(vt)
                for ntile in range(NT):
                    psv = psA.tile([128, 512], F32, tag="psA", name="psV")
                    for dt in range(DT):
                        nc.tensor.matmul(
                            psv[:, 0:256],
                            vts[dt][:, ntile * 128:(ntile + 1) * 128],
                            wv_sb[:, dt, :],
                            start=(dt == 0), stop=(dt == DT - 1),
                        )
                    nc.vector.tensor_copy(
                        vo_sb[:, ntile, :, 0:DH],
                        psv[:, 0:256].rearrange("p (h d) -> p h d", h=HPC),
                    )

            # ---- Phase B: attention, qh-outer so phase C's first half
            # ---- can interleave with the second q-half's attention.
            with (
                tc.tile_pool(name="psB", bufs=2, space="PSUM") as psB_scores,
                tc.tile_pool(name="psO", bufs=2, space="PSUM") as psO,
            ):
                def emit_c_block(qt, tail=False):
                    """One output-projection tile: y[qt*128:...] both e-halves.
                    PSUM comes from the scores pool (tag st) - no spare banks.
                    In the tail (nothing else running) spread the PSUM->SBUF
                    copies and y DMAs across engines/queues."""
                    for e in range(2):
                        py = psB_scores.tile([128, 1024], F32, tag="st",
                                             name="py")
                        for t in range(2):
                            nc.tensor.matmul(
                                py[:, 0:512],
                                aT_sb[:, t, qt * 128:(qt + 1) * 128],
                                wo_sb[:, t, e * 512:(e + 1) * 512],
                                start=(t == 0), stop=(t == 1),
                            )
                        ot = ysb_pool.tile([128, 512], BF16, tag="ysb",
                                           name="ysb")
                        if tail and e == 1:
                            nc.scalar.copy(ot, py[:, 0:512])
                        else:
                            nc.vector.tensor_copy(ot, py[:, 0:512])
                        dma_q = (nc.sync if (tail and qt % 2 == 0)
                                 else nc.gpsimd)
                        dma_q.dma_start(
                            y_d[qt * 128:(qt + 1) * 128,
                                e * 512:(e + 1) * 512], ot)

                def norm_head(h, qh0, oT, fast_tail=False):
                    """A^T[head rows] = O^T * (1/rowsum).
                    1/rowsum on DVE (fast approx, one pass over [1,1024]),
                    broadcast across the 64 dh partitions with a stride-0
                    SBUF->SBUF DMA, then one DVE multiply."""
                    t, p64 = h // 2, (h % 2) * 64
                    # Free the oT PSUM buffer early (two SBUF copies) so the
                    # next pair's first attnV doesn't wait the whole norm
                    # chain for the psO pool rotation.
                    oc = norm_pool.tile([64, 1024], F32, tag="oc", name="oc")
                    nc.vector.tensor_copy(oc, oT[0:64, :])
                    # custom-DVE ops misread nonzero base partitions on HW
                    # (CoreSim allows it): stage the rowsum row to partition 0
                    # before the reciprocal.
                    rsrow = norm_pool.tile([1, 1024], F32, tag="rsrow",
                                           name="rsrow")
                    nc.vector.tensor_copy(rsrow, oT[64:65, :])
                    rec = norm_pool.tile([1, 1024], F32, tag="rec",
                                         name="rec")
                    nc.vector.reciprocal_approx_fast(rec, rsrow)
                    if fast_tail:
                        # last pair: skip the two DRAM-bounce DMAs (their
                        # ~5us latency is fully exposed at the tail) and
                        # broadcast 1/rowsum across partitions with a rank-1
                        # matmul instead; the multiply reads oc from SBUF so
                        # rb may stay in PSUM.
                        recb = norm_pool.tile([1, 1024], BF16, tag="recb",
                                              name="recb")
                        nc.vector.tensor_copy(recb, rec)
                        rb = psB_scores.tile([128, 1024], F32, tag="st",
                                             name="rb")
                        for hf in range(2):
                            nc.tensor.matmul(
                                rb[0:64, hf * 512:(hf + 1) * 512],
                                ones_sb[0:1, :],
                                recb[:, hf * 512:(hf + 1) * 512],
                                start=True, stop=True)
                        rbs = rb[0:64, :]
                    else:
                        i = h * 2 + qh0 // 1024
                        rrow = rrec_d[i:i + 1, :]
                        nc.gpsimd.dma_start(rrow, rec)
                        rbs = norm_pool.tile([64, 1024], F32, tag="rbs",
                                             name="rbs")
                        bcast = bass.AP(tensor=rrow.tensor, offset=rrow.offset,
                                        ap=[[0, 64]] + [list(rrow.ap[-1])])
                        nc.gpsimd.dma_start(rbs, bcast)
                    if p64 == 0:
                        nc.vector.tensor_mul(
                            aT_sb[0:64, t, qh0:qh0 + 1024], oc, rbs)
                    else:
                        stage = norm_pool.tile([64, 1024], BF16, tag="stage",
                                               name="stage")
                        nc.vector.tensor_mul(stage, oc, rbs)
                        nc.gpsimd.dma_start(
                            aT_sb[64:128, t, qh0:qh0 + 1024], stage)

                c_emitted = 0
                kt_count = 0
                for qh in range(2):
                    qh0 = qh * 1024
                    nkt = (qh0 + 1024) // 128
                    # two heads per iteration (same kT/qT tile t, partition
                    # halves 0/64): the second stream fills PE bubbles while
                    # ScalarE runs the first stream's exp, and vice versa.
                    for pair in range(2):
                        t = pair
                        oTs = [psO.tile([65, 1024], F32, tag="oT",
                                        name=f"oT{s}") for s in range(2)]

                        def attn_v(kt, pTs, qs, oTs=oTs, qh0=qh0, t=t):
                            for s in range(2):
                                h = 2 * t + s
                                for qc in range(2):
                                    c0 = qh0 + qc * 512
                                    lo = max(qs, c0)
                                    if lo >= c0 + 512:
                                        continue
                                    ktl = (c0 + 512) // 128 - 1
                                    nc.tensor.matmul(
                                        oTs[s][:, qc * 512 + (lo - c0):(qc + 1) * 512],
                                        vo_sb[:, kt, h, :],
                                        pTs[s][:, lo - qh0:c0 + 512 - qh0],
                                        start=(kt == 0), stop=(kt == ktl),
                                    )

                        pending = []
                        for kt in range(nkt):
                            k0 = kt * 128
                            qs = max(k0, qh0)
                            off = qs - qh0
                            pTs = []
                            for s in range(2):
                                p64 = s * 64
                                st = psB_scores.tile([128, 1024], F32,
                                                     tag="st", name="st")
                                for bank in range(2):
                                    glo = max(qs, qh0 + bank * 512)
                                    ghi = qh0 + (bank + 1) * 512
                                    if glo < ghi:
                                        nc.tensor.matmul(
                                            st[:, glo - qh0:ghi - qh0],
                                            kT_sb[p64:p64 + 64, t, k0:k0 + 128],
                                            qT_sb[p64:p64 + 64, t, glo:ghi],
                                            start=True, stop=True,
                                        )
                                pT_t = pT_pool.tile([128, 1024], BF16,
                                                    tag="pT", name="pT")
                                nc.scalar.activation(
                                    pT_t[:, off:1024], st[:, off:1024], Exp,
                                    scale=0.125)
                                if k0 >= qh0:
                                    # gpsimd, not DVE: keeps the mask multiply
                                    # off the DVE queue (norm + cast traffic)
                                    # so attnV never waits behind those.
                                    nc.gpsimd.tensor_mul(
                                        pT_t[:, off:off + 128],
                                        pT_t[:, off:off + 128], tri_sb)
                                pTs.append(pT_t)
                            if dbg and qh == 0 and pair == 0 and kt == 2:
                                dstt = norm_pool.tile([128, 1024], F32,
                                                      tag="dst", name="dst")
                                nc.vector.tensor_copy(dstt, st)
                                nc.sync.dma_start(dst_d, dstt)
                                nc.sync.dma_start(dpT_d, pTs[1])
                            pending.append((kt, pTs, qs))
                            if len(pending) > 2:
                                attn_v(*pending.pop(0))
                            if qh == 1:
                                kt_count += 1
                                if (kt_count % 4 == 0 and c_emitted < 8):
                                    emit_c_block(c_emitted)
                                    c_emitted += 1
                        for p_ in pending:
                            attn_v(*p_)

                        if dbg and qh == 0 and pair == 0:
                            for s in range(2):
                                doTt = norm_pool.tile([65, 1024], F32,
                                                      tag="doT", name="doT")
                                nc.vector.tensor_copy(doTt, oTs[s])
                                nc.sync.dma_start(doT_d[s], doTt)

                        for s in range(2):
                            norm_head(2 * t + s, qh0, oTs[s],
                                      fast_tail=(qh == 1 and pair == 1))

                if dbg:
                    nc.sync.dma_start(dqT_d, qT_sb)
                    nc.sync.dma_start(dkT_d, kT_sb)
                    nc.sync.dma_start(dvo_d, vo_sb)
                    nc.sync.dma_start(daT_d, aT_sb)

                for qt in range(c_emitted, NT):
                    emit_c_block(qt, tail=True)

    nc.compile()
    return nc


_CACHE = {}
last_in_maps = None


def _get_program():
    if "nc" not in _CACHE:
        _CACHE["nc"] = build_program()
    return _CACHE["nc"]


def kernel(query, key, value, mask, Wq, Wk, Wv, Wo, bo):
    query = np.asarray(query, np.float32)
    key = np.asarray(key, np.float32)
    value = np.asarray(value, np.float32)
    Wq = np.asarray(Wq, np.float32)
    Wk = np.asarray(Wk, np.float32)
    Wv = np.asarray(Wv, np.float32)
    Wo = np.asarray(Wo, np.float32)
    bo = np.asarray(bo, np.float32)

    nc = _get_program()
    tri = np.ascontiguousarray(np.triu(np.ones((128, 128), np.float32)))
    WoT = Wo.T  # (d_in, d_out)

    in_maps = []
    for c in range(N_CORES):
        b, g = divmod(c, 4)
        hs = slice(g * HD, (g + 1) * HD)
        in_maps.append({
            "qT": np.ascontiguousarray(query[b].T).astype(ml_dtypes.bfloat16),
            "kT": np.ascontiguousarray(key[b].T).astype(ml_dtypes.bfloat16),
            "vT": np.ascontiguousarray(value[b].T).astype(ml_dtypes.bfloat16),
            "wq": np.ascontiguousarray(Wq[hs].T).astype(ml_dtypes.bfloat16),
            "wk": np.ascontiguousarray(Wk[hs].T).astype(ml_dtypes.bfloat16),
            "wv": np.ascontiguousarray(Wv[hs].T).astype(ml_dtypes.bfloat16),
            "wo": np.ascontiguousarray(WoT[hs]).astype(ml_dtypes.bfloat16),
            "tri": tri.astype(ml_dtypes.bfloat16),
            "ones": np.ones((128, 64), ml_dtypes.bfloat16),
        })

    global last_in_maps
    last_in_maps = in_maps
    res = run_bass_kernel_spmd(nc, in_maps, core_ids=list(range(N_CORES)))

    out = np.zeros((B, S, D), np.float32)
    for c in range(N_CORES):
        out[c // 4] += res.results[c]["y"].astype(np.float32)
    out += bo
    return out


# revision 52
# speedup vs baseline: 1.0508x; 1.0508x over previous
"""Multi-head causal attention (B=2, S=2048, D=1024, H=16) on 8 NeuronCores.

Sharding: core c handles batch c//4 and head group c%4 (4 heads = 256 dims).
Wq/Wk/Wv are column-split by head (each core projects only its heads);
Wo is row-split; the Wo all-reduce is done on the host by summing the 8
partial outputs (4 cores per batch), then adding bo.

Per-core device program (all matmuls in bf16, accumulation in f32 PSUM):
  A) Q/K projections -> Q^T, K^T laid out (head_dim, seq) in SBUF;
     V projection -> natural (seq, head_dim) with a ones column appended
     (so attn@V also produces the softmax row-sums as a 65th output row).
  B) Per (head, q-half): scores computed transposed (k on partitions,
     q on free dim) -> exp on ScalarE (no max subtraction; scores ~N(0,1))
     -> attn@V accumulating O^T (65, q) in PSUM. Causal masking = skip
     fully-masked column ranges in the matmuls + one triangular 128x128
     mask multiply per diagonal k-tile. Normalization: DVE fast reciprocal
     of the PSUM row-sum row -> rank-1 matmul (ones[1,64] x rec[1,1024])
     broadcasts it across partitions in PSUM -> DVE multiply.
  C) Output projection from the accumulated A^T (256, seq) against
     Wo^T slice, streamed out per 128-row tile as bf16.
"""

import ml_dtypes
import numpy as np

import concourse.bass as bass
import concourse.tile as tile
from concourse import bacc, mybir
from concourse.bass_utils import run_bass_kernel_spmd

B, S, D, H = 2, 2048, 1024, 16
DH = D // H          # 64
HPC = 4              # heads per core
HD = HPC * DH        # 256 head dims per core
N_CORES = 8
DT = D // 128        # 8 contraction tiles for projections
NT = S // 128        # 16 seq tiles
F32 = mybir.dt.float32
F32R = mybir.dt.float32r
BF16 = mybir.dt.bfloat16
F8 = mybir.dt.float8e4
DR = mybir.MatmulPerfMode.DoubleRow


def build_program(dbg=False, num_devices=N_CORES):
    nc = bacc.Bacc("TRN2", target_bir_lowering=False, debug=False,
                   num_devices=num_devices)

    qT_d = nc.dram_tensor("qT", [D, S], BF16, kind="ExternalInput").ap()
    kT_d = nc.dram_tensor("kT", [D, S], BF16, kind="ExternalInput").ap()
    vT_d = nc.dram_tensor("vT", [D, S], BF16, kind="ExternalInput").ap()
    wq_d = nc.dram_tensor("wq", [D, HD], BF16, kind="ExternalInput").ap()
    wk_d = nc.dram_tensor("wk", [D, HD], BF16, kind="ExternalInput").ap()
    wv_d = nc.dram_tensor("wv", [D, HD], BF16, kind="ExternalInput").ap()
    wo_d = nc.dram_tensor("wo", [HD, D], BF16, kind="ExternalInput").ap()
    tri_d = nc.dram_tensor("tri", [128, 128], BF16, kind="ExternalInput").ap()
    ones_d = nc.dram_tensor("ones", [128, 64], BF16,
                            kind="ExternalInput").ap()
    # DRAM staging for the reciprocal rowsum rows: a stride-0 partition
    # broadcast is only legal when reading from DRAM.
    rrec_d = nc.dram_tensor("rrec", [8, 1024], F32,
                            kind="ExternalOutput" if dbg else "Internal").ap()
    y_d = nc.dram_tensor("y", [S, D], BF16, kind="ExternalOutput").ap()
    if dbg:
        dqT_d = nc.dram_tensor("dqT", [128, 2, S], BF16, kind="ExternalOutput").ap()
        dkT_d = nc.dram_tensor("dkT", [128, 2, S], BF16, kind="ExternalOutput").ap()
        dvo_d = nc.dram_tensor("dvo", [128, NT, HPC, DH + 1], BF16,
                               kind="ExternalOutput").ap()
        daT_d = nc.dram_tensor("daT", [128, 2, S], BF16, kind="ExternalOutput").ap()
        dst_d = nc.dram_tensor("dst", [128, 1024], F32, kind="ExternalOutput").ap()
        dpT_d = nc.dram_tensor("dpT", [128, 1024], BF16, kind="ExternalOutput").ap()
        doT_d = nc.dram_tensor("doT", [2, 65, 1024], F32, kind="ExternalOutput").ap()

    Exp = mybir.ActivationFunctionType.Exp

    with tile.TileContext(nc) as tc:
        with (
            tc.tile_pool(name="persist", bufs=1) as persist,
            tc.tile_pool(name="stream", bufs=4) as qstream,
            tc.tile_pool(name="kstream", bufs=4) as kstream,
            tc.tile_pool(name="vstream", bufs=8) as vstream,
            tc.tile_pool(name="pT", bufs=6) as pT_pool,
            tc.tile_pool(name="norm", bufs=2) as norm_pool,
            tc.tile_pool(name="ysb", bufs=4) as ysb_pool,
        ):
            # ---- constants / weights (one DMA each, spread over queues) ----
            # All matmuls bf16: fp8 anywhere in these dot-product chains
            # costs ~4% relative error (random-sign sums don't average the
            # quantization noise away) and fails the accuracy gate.
            wq_sb = persist.tile([128, DT, HD], BF16, tag="wq")
            wk_sb = persist.tile([128, DT, HD], BF16, tag="wk")
            wv_sb = persist.tile([128, DT, HD], BF16, tag="wv")
            wo_sb = persist.tile([128, 2, D], BF16, tag="wo")
            tri_sb = persist.tile([128, 128], BF16, tag="tri")
            ones_sb = persist.tile([128, 64], BF16, tag="ones")

            # Weights on the scalar queue; x streams go on the sync queue
            # staggered q -> k -> v so each tensor gets full HBM bandwidth
            # just before its projection needs it (concurrent streams would
            # starve Q proj).
            # wq dt=0 slice first: the very first matmul needs only it, so
            # don't make it wait for the full 0.5MB load.
            wq_r = wq_d.rearrange("(t p) h -> p t h", p=128)
            nc.scalar.dma_start(wq_sb[:, 0:1, :], wq_r[:, 0:1, :])
            nc.scalar.dma_start(wq_sb[:, 1:, :], wq_r[:, 1:, :])
            nc.scalar.dma_start(wk_sb, wk_d.rearrange("(t p) h -> p t h", p=128))
            nc.scalar.dma_start(wv_sb, wv_d.rearrange("(t p) h -> p t h", p=128))
            nc.scalar.dma_start(tri_sb, tri_d)
            nc.scalar.dma_start(ones_sb, ones_d)
            nc.scalar.dma_start(wo_sb, wo_d.rearrange("(t p) e -> p t e", p=128))

            qT_sb = persist.tile([128, 2, S], BF16, tag="qTsb")
            kT_sb = persist.tile([128, 2, S], BF16, tag="kTsb")
            # V with ones column: [128, seq_tile, head, 65]
            vo_sb = persist.tile([128, NT, HPC, DH + 1], BF16, tag="vones")
            nc.vector.tensor_copy(
                vo_sb[:, :, :, DH:DH + 1],
                ones_sb.rearrange("p (t h one) -> p t h one", h=HPC, one=1))
            aT_sb = persist.tile([128, 2, S], BF16, tag="aTsb")

            # ---- Phase A: projections (fp8 DoubleRow: 2 contraction
            # ---- subtiles per pass, 0.5 cycles per moving column) ----
            with tc.tile_pool(name="psA", bufs=8, space="PSUM") as psA:
                def proj_qk(x_d, w_sb, dst_sb, pool):
                    ps = [psA.tile([128, 512], F32, tag="psA", name=f"psA{i}")
                          for i in range(8)]
                    for dt in range(DT):
                        xt = pool.tile([128, S], BF16)
                        nc.sync.dma_start(xt, x_d[dt * 128:(dt + 1) * 128, :])
                        for half in range(2):
                            for t in range(2):
                                for c in range(2):
                                    cc = half * 2 + c
                                    nc.tensor.matmul(
                                        ps[t * 4 + cc],
                                        w_sb[:, dt, t * 128:(t + 1) * 128],
                                        xt[:, half * 1024 + c * 512:
                                           half * 1024 + (c + 1) * 512],
                                        start=(dt == 0), stop=(dt == DT - 1),
                                    )
                    for t in range(2):
                        for c in range(4):
                            nc.vector.tensor_copy(
                                dst_sb[:, t, c * 512:(c + 1) * 512],
                                ps[t * 4 + c])

                proj_qk(qT_d, wq_sb, qT_sb, qstream)
                proj_qk(kT_d, wk_sb, kT_sb, kstream)

                # V: natural layout out[m = seq_tile(128), n = head dims(256)].
                # ntile-outer so each PSUM tile's copy issues right after its
                # accumulation matmuls (no bunched drain at the A->B edge).
                vts = []
                for dt in range(DT):
                    vt = vstream.tile([128, S], BF16)
                    nc.sync.dma_start(vt, vT_d[dt * 128:(dt + 1) * 128, :])
                    vts.append(vt)
                for ntile in range(NT):
                    psv = psA.tile([128, 512], F32, tag="psA", name="psV")
                    for dt in range(DT):
                        nc.tensor.matmul(
                            psv[:, 0:256],
                            vts[dt][:, ntile * 128:(ntile + 1) * 128],
                            wv_sb[:, dt, :],
                            start=(dt == 0), stop=(dt == DT - 1),
                        )
                    nc.vector.tensor_copy(
                        vo_sb[:, ntile, :, 0:DH],
                        psv[:, 0:256].rearrange("p (h d) -> p h d", h=HPC),
                    )

            # ---- Phase B: attention, qh-outer so phase C's first half
            # ---- can interleave with the second q-half's attention.
            with (
                tc.tile_pool(name="psB", bufs=2, space="PSUM") as psB_scores,
                tc.tile_pool(name="psO", bufs=2, space="PSUM") as psO,
            ):
                def emit_c_block(qt, tail=False):
                    """One output-projection tile: y[qt*128:...] both e-halves.
                    PSUM comes from the scores pool (tag st) - no spare banks.
                    In the tail (nothing else running) spread the PSUM->SBUF
                    copies and y DMAs across engines/queues."""
                    for e in range(2):
                        py = psB_scores.tile([128, 1024], F32, tag="st",
                                             name="py")
                        for t in range(2):
                            nc.tensor.matmul(
                                py[:, 0:512],
                                aT_sb[:, t, qt * 128:(qt + 1) * 128],
                                wo_sb[:, t, e * 512:(e + 1) * 512],
                                start=(t == 0), stop=(t == 1),
                            )
                        ot = ysb_pool.tile([128, 512], BF16, tag="ysb",
                                           name="ysb")
                        if tail and e == 1:
                            nc.scalar.copy(ot, py[:, 0:512])
                        else:
                            nc.vector.tensor_copy(ot, py[:, 0:512])
                        dma_q = (nc.sync if (tail and qt % 2 == 0)
                                 else nc.gpsimd)
                        dma_q.dma_start(
                            y_d[qt * 128:(qt + 1) * 128,
                                e * 512:(e + 1) * 512], ot)

                def norm_head(h, qh0, oT, fast_tail=False):
                    """A^T[head rows] = O^T * (1/rowsum).
                    1/rowsum on DVE (fast approx, one pass over [1,1024]),
                    broadcast across the 64 dh partitions with a stride-0
                    SBUF->SBUF DMA, then one DVE multiply."""
                    t, p64 = h // 2, (h % 2) * 64
                    # Free the oT PSUM buffer early (two SBUF copies) so the
                    # next pair's first attnV doesn't wait the whole norm
                    # chain for the psO pool rotation.
                    oc = norm_pool.tile([64, 1024], F32, tag="oc", name="oc")
                    nc.vector.tensor_copy(oc, oT[0:64, :])
                    # custom-DVE ops misread nonzero base partitions on HW
                    # (CoreSim allows it): stage the rowsum row to partition 0
                    # before the reciprocal.
                    rsrow = norm_pool.tile([1, 1024], F32, tag="rsrow",
                                           name="rsrow")
                    nc.vector.tensor_copy(rsrow, oT[64:65, :])
                    rec = norm_pool.tile([1, 1024], F32, tag="rec",
                                         name="rec")
                    nc.vector.reciprocal_approx_fast(rec, rsrow)
                    if fast_tail:
                        # last pair: skip the two DRAM-bounce DMAs (their
                        # ~5us latency is fully exposed at the tail) and
                        # broadcast 1/rowsum across partitions with a rank-1
                        # matmul instead; the multiply reads oc from SBUF so
                        # rb may stay in PSUM.
                        recb = norm_pool.tile([1, 1024], BF16, tag="recb",
                                              name="recb")
                        nc.vector.tensor_copy(recb, rec)
                        rb = psB_scores.tile([128, 1024], F32, tag="st",
                                             name="rb")
                        for hf in range(2):
                            nc.tensor.matmul(
                                rb[0:64, hf * 512:(hf + 1) * 512],
                                ones_sb[0:1, :],
                                recb[:, hf * 512:(hf + 1) * 512],
                                start=True, stop=True)
                        rbs = rb[0:64, :]
                    else:
                        i = h * 2 + qh0 // 1024
                        rrow = rrec_d[i:i + 1, :]
                        nc.gpsimd.dma_start(rrow, rec)
                        rbs = norm_pool.tile([64, 1024], F32, tag="rbs",
                                             name="rbs")
                        bcast = bass.AP(tensor=rrow.tensor, offset=rrow.offset,
                                        ap=[[0, 64]] + [list(rrow.ap[-1])])
                        nc.gpsimd.dma_start(rbs, bcast)
                    if p64 == 0:
                        nc.vector.tensor_mul(
                            aT_sb[0:64, t, qh0:qh0 + 1024], oc, rbs)
                    else:
                        stage = norm_pool.tile([64, 1024], BF16, tag="stage",
                                               name="stage")
                        nc.vector.tensor_mul(stage, oc, rbs)
                        nc.gpsimd.dma_start(
                            aT_sb[64:128, t, qh0:qh0 + 1024], stage)

                c_emitted = 0
                kt_count = 0
                for qh in range(2):
                    qh0 = qh * 1024
                    nkt = (qh0 + 1024) // 128
                    # two heads per iteration (same kT/qT tile t, partition
                    # halves 0/64): the second stream fills PE bubbles while
                    # ScalarE runs the first stream's exp, and vice versa.
                    for pair in range(2):
                        t = pair
                        oTs = [psO.tile([65, 1024], F32, tag="oT",
                                        name=f"oT{s}") for s in range(2)]

                        def attn_v(kt, pTs, qs, oTs=oTs, qh0=qh0, t=t):
                            for s in range(2):
                                h = 2 * t + s
                                for qc in range(2):
                                    c0 = qh0 + qc * 512
                                    lo = max(qs, c0)
                                    if lo >= c0 + 512:
                                        continue
                                    ktl = (c0 + 512) // 128 - 1
                                    nc.tensor.matmul(
                                        oTs[s][:, qc * 512 + (lo - c0):(qc + 1) * 512],
                                        vo_sb[:, kt, h, :],
                                        pTs[s][:, lo - qh0:c0 + 512 - qh0],
                                        start=(kt == 0), stop=(kt == ktl),
                                    )

                        pending = []
                        for kt in range(nkt):
                            k0 = kt * 128
                            qs = max(k0, qh0)
                            off = qs - qh0
                            pTs = []
                            for s in range(2):
                                p64 = s * 64
                                st = psB_scores.tile([128, 1024], F32,
                                                     tag="st", name="st")
                                for bank in range(2):
                                    glo = max(qs, qh0 + bank * 512)
                                    ghi = qh0 + (bank + 1) * 512
                                    if glo < ghi:
                                        nc.tensor.matmul(
                                            st[:, glo - qh0:ghi - qh0],
                                            kT_sb[p64:p64 + 64, t, k0:k0 + 128],
                                            qT_sb[p64:p64 + 64, t, glo:ghi],
                                            start=True, stop=True,
                                        )
                                pT_t = pT_pool.tile([128, 1024], BF16,
                                                    tag="pT", name="pT")
                                nc.scalar.activation(
                                    pT_t[:, off:1024], st[:, off:1024], Exp,
                                    scale=0.125)
                                if k0 >= qh0:
                                    # gpsimd, not DVE: keeps the mask multiply
                                    # off the DVE queue (norm + cast traffic)
                                    # so attnV never waits behind those.
                                    nc.gpsimd.tensor_mul(
                                        pT_t[:, off:off + 128],
                                        pT_t[:, off:off + 128], tri_sb)
                                pTs.append(pT_t)
                            if dbg and qh == 0 and pair == 0 and kt == 2:
                                dstt = norm_pool.tile([128, 1024], F32,
                                                      tag="dst", name="dst")
                                nc.vector.tensor_copy(dstt, st)
                                nc.sync.dma_start(dst_d, dstt)
                                nc.sync.dma_start(dpT_d, pTs[1])
                            pending.append((kt, pTs, qs))
                            if len(pending) > 2:
                                attn_v(*pending.pop(0))
                            if qh == 1:
                                kt_count += 1
                                if (kt_count % 4 == 0 and c_emitted < 8):
                                    emit_c_block(c_emitted)
                                    c_emitted += 1
                        for p_ in pending:
                            attn_v(*p_)

                        if dbg and qh == 0 and pair == 0:
                            for s in range(2):
                                doTt = norm_pool.tile([65, 1024], F32,
                                                      tag="doT", name="doT")
                                nc.vector.tensor_copy(doTt, oTs[s])
                                nc.sync.dma_start(doT_d[s], doTt)

                        for s in range(2):
                            norm_head(2 * t + s, qh0, oTs[s],
                                      fast_tail=(qh == 1 and pair == 1))

                if dbg:
                    nc.sync.dma_start(dqT_d, qT_sb)
                    nc.sync.dma_start(dkT_d, kT_sb)
                    nc.sync.dma_start(dvo_d, vo_sb)
                    nc.sync.dma_start(daT_d, aT_sb)

                for qt in range(c_emitted, NT):
                    emit_c_block(qt, tail=True)

    nc.compile()
    return nc


_CACHE = {}
last_in_maps = None


def _get_program():
    if "nc" not in _CACHE:
        _CACHE["nc"] = build_program()
    return _CACHE["nc"]


def kernel(query, key, value, mask, Wq, Wk, Wv, Wo, bo):
    query = np.asarray(query, np.float32)
    key = np.asarray(key, np.float32)
    value = np.asarray(value, np.float32)
    Wq = np.asarray(Wq, np.float32)
    Wk = np.asarray(Wk, np.float32)
    Wv = np.asarray(Wv, np.float32)
    Wo = np.asarray(Wo, np.float32)
    bo = np.asarray(bo, np.float32)

    nc = _get_program()
    tri = np.ascontiguousarray(np.triu(np.ones((128, 128), np.float32)))
    WoT = Wo.T  # (d_in, d_out)

    in_maps = []
    for c in range(N_CORES):
        b, g = divmod(c, 4)
        hs = slice(g * HD, (g + 1) * HD)
        in_maps.append({
            "qT": np.ascontiguousarray(query[b].T).astype(ml_dtypes.bfloat16),
            "kT": np.ascontiguousarray(key[b].T).astype(ml_dtypes.bfloat16),
            "vT": np.ascontiguousarray(value[b].T).astype(ml_dtypes.bfloat16),
            "wq": np.ascontiguousarray(Wq[hs].T).astype(ml_dtypes.bfloat16),
            "wk": np.ascontiguousarray(Wk[hs].T).astype(ml_dtypes.bfloat16),
            "wv": np.ascontiguousarray(Wv[hs].T).astype(ml_dtypes.bfloat16),
            "wo": np.ascontiguousarray(WoT[hs]).astype(ml_dtypes.bfloat16),
            "tri": tri.astype(ml_dtypes.bfloat16),
            "ones": np.ones((128, 64), ml_dtypes.bfloat16),
        })

    global last_in_maps
    last_in_maps = in_maps
    res = run_bass_kernel_spmd(nc, in_maps, core_ids=list(range(N_CORES)))

    out = np.zeros((B, S, D), np.float32)
    for c in range(N_CORES):
        out[c // 4] += res.results[c]["y"].astype(np.float32)
    out += bo
    return out
